# revision 1
# baseline (speedup 1.0000x reference)
"""Trainium2 Bass kernel for nn_DetectionLoss (MSE + cost-sensitive log term).

Contract: kernel(outputs, labels) takes the FULL [64, 1000000] float32 inputs,
shards them row-wise across 8 NeuronCores (8 rows per core), computes per-shard
partial sums on-device, and combines them on the host into the scalar loss:

    mse  = mean((outputs - labels)^2)
    pred = outputs > 0.5
    TP   = sum(labels * pred);  FN = sum(labels * (1 - pred))
    coeff = 1 if TP==0 and FN==0 else (0 if TP==0 else TP/(TP+FN))
    loss = mse + 0.5 * (-log(coeff + 1e-10))

Per-core device work (memory-bound, one streaming pass over both tensors):
    st[0] = sum(o^2)          (ScalarE Square + fused free-axis accumulate)
    st[1] = sum(l)            (ScalarE Identity + accumulate)
    st[2] = sum((o<=0.5)*l)   (VectorE scalar_tensor_tensor + accumulate) == FN
    st[3] = sum(o*l)          (VectorE scalar_tensor_tensor + accumulate)
Then sse = sum(o^2) - 2*sum(o*l) + sum(l) (since l in {0,1} => l^2 == l) and
TP = sum(l) - FN, combined in float64 on the host.
"""
import sys

import numpy as np

try:
    import concourse.bacc as bacc
except ImportError:  # pragma: no cover - fallback for bare environments
    sys.path.insert(0, "/opt/trn_rl_repo")
    import concourse.bacc as bacc

import concourse.tile as tile
from concourse import mybir
from concourse.bass_utils import run_bass_kernel_spmd

N_CORES = 8
ROWS, COLS = 64, 1000000          # full input shape
RPC = ROWS // N_CORES             # rows per core = 8
P = 128                           # SBUF partitions
NCOL = RPC * COLS // P            # 62500 free elements per partition per shard
F = 6250                          # tile free dim
T = NCOL // F                     # 10 tiles
LAMBD = 0.5
EPS = 1e-10

_nc_cache = None


def _build():
    f32 = mybir.dt.float32
    nc = bacc.Bacc("TRN2", target_bir_lowering=False, debug=False,
                   num_devices=N_CORES)
    o = nc.dram_tensor("outputs", [P, NCOL], f32, kind="ExternalInput").ap()
    l = nc.dram_tensor("labels", [P, NCOL], f32, kind="ExternalInput").ap()
    st = nc.dram_tensor("stats", [4, P, T], f32, kind="ExternalOutput").ap()

    with tile.TileContext(nc) as tc:
        with (
            tc.tile_pool(name="io", bufs=2) as io_pool,
            tc.tile_pool(name="scratch", bufs=1) as sp,
            tc.tile_pool(name="stats", bufs=1) as stp,
        ):
            sq_st = stp.tile([P, T], f32, tag="sq")
            l_st = stp.tile([P, T], f32, tag="l")
            fn_st = stp.tile([P, T], f32, tag="fn")
            ol_st = stp.tile([P, T], f32, tag="ol")
            dve_scr = sp.tile([P, F], f32, tag="dve")
            act_scr = sp.tile([P, F], f32, tag="act")
            for t in range(T):
                ot = io_pool.tile([P, F], f32, tag="o")
                lt = io_pool.tile([P, F], f32, tag="lb")
                nc.sync.dma_start(ot[:], o[:, t * F:(t + 1) * F])
                nc.sync.dma_start(lt[:], l[:, t * F:(t + 1) * F])
                # FN partial: (o <= 0.5) * l, summed over the free axis
                nc.vector.scalar_tensor_tensor(
                    out=dve_scr[:], in0=ot[:], scalar=0.5, in1=lt[:],
                    op0=mybir.AluOpType.is_le, op1=mybir.AluOpType.mult,
                    accum_out=fn_st[:, t:t + 1],
                )
                # sum(o*l) partial via (o*1.0)*l
                nc.vector.scalar_tensor_tensor(
                    out=dve_scr[:], in0=ot[:], scalar=1.0, in1=lt[:],
                    op0=mybir.AluOpType.mult, op1=mybir.AluOpType.mult,
                    accum_out=ol_st[:, t:t + 1],
                )
                # sum(o^2) partial
                nc.scalar.activation(
                    out=act_scr[:], in_=ot[:],
                    func=mybir.ActivationFunctionType.Square,
                    accum_out=sq_st[:, t:t + 1],
                )
                # sum(l) partial
                nc.scalar.activation(
                    out=act_scr[:], in_=lt[:],
                    func=mybir.ActivationFunctionType.Identity,
                    accum_out=l_st[:, t:t + 1],
                )
            nc.sync.dma_start(st[0], sq_st[:])
            nc.sync.dma_start(st[1], l_st[:])
            nc.sync.dma_start(st[2], fn_st[:])
            nc.sync.dma_start(st[3], ol_st[:])
    nc.compile()
    return nc


def _get_nc():
    global _nc_cache
    if _nc_cache is None:
        _nc_cache = _build()
    return _nc_cache


def _run(outputs, labels, trace=False):
    assert outputs.shape == (ROWS, COLS) and labels.shape == (ROWS, COLS)
    outputs = np.ascontiguousarray(outputs, dtype=np.float32)
    labels = np.ascontiguousarray(labels, dtype=np.float32)
    in_maps = [
        {
            "outputs": outputs[c * RPC:(c + 1) * RPC].reshape(P, NCOL),
            "labels": labels[c * RPC:(c + 1) * RPC].reshape(P, NCOL),
        }
        for c in range(N_CORES)
    ]
    nc = _get_nc()
    res = run_bass_kernel_spmd(nc, in_maps, list(range(N_CORES)), trace=trace)
    stats = np.stack([res.results[c]["stats"] for c in range(N_CORES)])
    s = stats.astype(np.float64).sum(axis=(0, 2, 3))  # [4]
    sum_sq, sum_l, fn, sum_ol = s
    sse = sum_sq - 2.0 * sum_ol + sum_l
    mse = sse / (ROWS * COLS)
    tp = sum_l - fn
    if tp == 0.0 and fn == 0.0:
        coeff = 1.0
    elif tp == 0.0:
        coeff = 0.0
    else:
        coeff = tp / (tp + fn)
    loss = mse + LAMBD * (-np.log(coeff + EPS))
    return np.float32(loss), res


def kernel(outputs, labels):
    val, _ = _run(outputs, labels)
    return val


# revision 2
# speedup vs baseline: 1.1065x; 1.1065x over previous
"""Trainium2 Bass kernel for nn_DetectionLoss (MSE + cost-sensitive log term).

Contract: kernel(outputs, labels) takes the FULL [64, 1000000] float32 inputs,
shards them row-wise across 8 NeuronCores (8 rows per core), computes per-shard
partial sums on-device, and combines them on the host into the scalar loss:

    mse  = mean((outputs - labels)^2)
    pred = outputs > 0.5
    TP   = sum(labels * pred);  FN = sum(labels * (1 - pred))
    coeff = 1 if TP==0 and FN==0 else (0 if TP==0 else TP/(TP+FN))
    loss = mse + 0.5 * (-log(coeff + 1e-10))

Per-core device work (memory-bound, one streaming pass over both tensors):
    st[0] = sum(o^2)          (ScalarE Square + fused free-axis accumulate)
    st[1] = sum(l)            (ScalarE Identity + accumulate)
    st[2] = sum((o<=0.5)*l)   (VectorE scalar_tensor_tensor + accumulate) == FN
    st[3] = sum(o*l)          (VectorE scalar_tensor_tensor + accumulate)
Then sse = sum(o^2) - 2*sum(o*l) + sum(l) (since l in {0,1} => l^2 == l) and
TP = sum(l) - FN, combined in float64 on the host.

Each core's two input shards are stacked host-side into one [128, 2, 62500]
tensor so every tile needs a single 3.2 MB DMA (one semaphore, big transfers
-> ~420 GB/s effective when the HBM stack isn't contended).
"""
import sys

import numpy as np

try:
    import concourse.bacc as bacc
except ImportError:  # pragma: no cover - fallback for bare environments
    sys.path.insert(0, "/opt/trn_rl_repo")
    import concourse.bacc as bacc

import concourse.tile as tile
from concourse import mybir
from concourse.bass_utils import run_bass_kernel_spmd

N_CORES = 8
ROWS, COLS = 64, 1000000          # full input shape
RPC = ROWS // N_CORES             # rows per core = 8
P = 128                           # SBUF partitions
NCOL = RPC * COLS // P            # 62500 free elements per partition per shard
F = 3125                          # tile free dim (3.2 MB per stacked tile DMA)
BUFS = 3
LAMBD = 0.5
EPS = 1e-10

_nc_cache = None


def _tiles():
    return [(t * F, F) for t in range(NCOL // F)]


def _build():
    f32 = mybir.dt.float32
    tiles = _tiles()
    nst = len(tiles)
    nc = bacc.Bacc("TRN2", target_bir_lowering=False, debug=False,
                   num_devices=N_CORES)
    x = nc.dram_tensor("x", [P, 2, NCOL], f32, kind="ExternalInput").ap()
    st = nc.dram_tensor("stats", [4, P, nst], f32, kind="ExternalOutput").ap()

    with tile.TileContext(nc) as tc:
        with (
            tc.tile_pool(name="io", bufs=BUFS) as io_pool,
            tc.tile_pool(name="scratch", bufs=1) as sp,
            tc.tile_pool(name="stats", bufs=1) as stp,
        ):
            sq_st = stp.tile([P, nst], f32, tag="sq")
            l_st = stp.tile([P, nst], f32, tag="l")
            fn_st = stp.tile([P, nst], f32, tag="fn")
            ol_st = stp.tile([P, nst], f32, tag="ol")
            dve_scr = sp.tile([P, F], f32, tag="dve")
            act_scr = sp.tile([P, F], f32, tag="act")
            for t, (c0, w) in enumerate(tiles):
                xt = io_pool.tile([P, 2, F], f32, tag="x")
                nc.sync.dma_start(xt[:, :, :w], x[:, :, c0:c0 + w])
                ot = xt[:, 0, :w]
                lt = xt[:, 1, :w]
                # FN partial: (o <= 0.5) * l, summed over the free axis
                nc.vector.scalar_tensor_tensor(
                    out=dve_scr[:, :w], in0=ot, scalar=0.5, in1=lt,
                    op0=mybir.AluOpType.is_le, op1=mybir.AluOpType.mult,
                    accum_out=fn_st[:, t:t + 1],
                )
                # sum(o*l) partial via (o*1.0)*l
                nc.vector.scalar_tensor_tensor(
                    out=dve_scr[:, :w], in0=ot, scalar=1.0, in1=lt,
                    op0=mybir.AluOpType.mult, op1=mybir.AluOpType.mult,
                    accum_out=ol_st[:, t:t + 1],
                )
                # sum(o^2) partial
                nc.scalar.activation(
                    out=act_scr[:, :w], in_=ot,
                    func=mybir.ActivationFunctionType.Square,
                    accum_out=sq_st[:, t:t + 1],
                )
                # sum(l) partial
                nc.scalar.activation(
                    out=act_scr[:, :w], in_=lt,
                    func=mybir.ActivationFunctionType.Identity,
                    accum_out=l_st[:, t:t + 1],
                )
            nc.sync.dma_start(st[0], sq_st[:])
            nc.sync.dma_start(st[1], l_st[:])
            nc.sync.dma_start(st[2], fn_st[:])
            nc.sync.dma_start(st[3], ol_st[:])
    nc.compile()
    return nc


def _get_nc():
    global _nc_cache
    if _nc_cache is None:
        _nc_cache = _build()
    return _nc_cache


def _run(outputs, labels, trace=False, **spmd_kwargs):
    assert outputs.shape == (ROWS, COLS) and labels.shape == (ROWS, COLS)
    outputs = np.ascontiguousarray(outputs, dtype=np.float32)
    labels = np.ascontiguousarray(labels, dtype=np.float32)
    in_maps = []
    for c in range(N_CORES):
        o = outputs[c * RPC:(c + 1) * RPC].reshape(P, NCOL)
        l = labels[c * RPC:(c + 1) * RPC].reshape(P, NCOL)
        in_maps.append({"x": np.stack([o, l], axis=1)})
    nc = _get_nc()
    res = run_bass_kernel_spmd(nc, in_maps, list(range(N_CORES)), trace=trace,
                               **spmd_kwargs)
    stats = np.stack([res.results[c]["stats"] for c in range(N_CORES)])
    s = stats.astype(np.float64).sum(axis=(0, 2, 3))  # [4]
    sum_sq, sum_l, fn, sum_ol = s
    sse = sum_sq - 2.0 * sum_ol + sum_l
    mse = sse / (ROWS * COLS)
    tp = sum_l - fn
    if tp == 0.0 and fn == 0.0:
        coeff = 1.0
    elif tp == 0.0:
        coeff = 0.0
    else:
        coeff = tp / (tp + fn)
    loss = mse + LAMBD * (-np.log(coeff + EPS))
    return np.float32(loss), res


def kernel(outputs, labels):
    val, _ = _run(outputs, labels)
    return val
